# revision 48
# baseline (speedup 1.0000x reference)
"""Trainium2 Bass kernel for nn_BaseLSTM: y = sigmoid(Dense(LSTM(x))).

Reference (per batch b, time t):
    xz = x @ Wx + b                      # [B,S,4H], keras gate order i,f,g,o
    z_t = xz_t + h_{t-1} @ Wh
    i,f,o = sigmoid(z_i), sigmoid(z_f), sigmoid(z_o);  g = tanh(z_g)
    c_t = f*c + i*g;  h_t = o * tanh(c_t)
    y_t = sigmoid(h_t @ Wd + bd)

Sharding: data-parallel over batch, 8 batches per core on 8 cores.

Per-core design (B_LOC=8 batches, split into pairs of coupled "chains"):
  - All matmuls in bf16 (fp32 PSUM accumulation).
  - tanh is computed as 2*sigmoid(2x)-1 so every transcendental is a sigmoid;
    the g-gate pre-activation is pre-scaled by 2 by folding the factor into
    Wx/Wh/b g-columns on the host.  The cell state is kept as c2 = 2*c so
    tanh(c) = 2*sigmoid(c2)-1 with no extra scaling op.
  - PSUM "window" layout per chain: bank tile [128, T_W*5*B_C] f32, per step
    slot [z_i | z_f | z_g | z_o | c2] (each B_C cols).  A window is pre-filled
    by PE matmuls: bias (K=1, rhs=ones) then Wx (K=64, rhs = transposed x
    window); the per-step Wh matmuls (K=128, rhs=h) accumulate on top.  The
    sigmoid over one slot [128, 5*B_C] then yields all four gates AND the
    partner chain's tanh argument in a single ACT instruction.
  - x is transposed via the DMA XBAR: x (f32, [S,64] per batch) is cast to
    bf16 into DRAM scratch Xp [S, 128] (two batches side by side), then
    dma_start_transpose lifts [T_W,128] -> [128,T_W] SBUF tiles per window.
  - h_t (bf16) is written by the DVE straight into a per-window SBUF tile
    that also serves as the next step's matmul rhs; windows are DMA'd to DRAM
    and re-read for the final Dense+sigmoid pass (PE K=128 M=1 matmul).
"""

import time
from contextlib import ExitStack

import ml_dtypes
import numpy as np

import concourse.bacc as bacc
import concourse.mybir as mybir
import concourse.tile as tile
from concourse import bass_utils

F32 = mybir.dt.float32
BF16 = mybir.dt.bfloat16
AF = mybir.ActivationFunctionType
OP = mybir.AluOpType

B, S_FULL, D, H = 64, 2048, 64, 128
NCORES = 8
B_LOC = B // NCORES  # 8
G = 4
T_W = 16             # steps per PSUM window
N_CHAINS = 0         # 0 = v2 fused-cell single chain; >=1 = legacy chains
EMIT_ORDER = "ab_offset"  # emission interleave of per-chain phases
HEAD_INTERLEAVE = True   # fold dense-head chunks into the recurrence stream


_IG_OP = None


def get_ig_op():
    """Custom DVE op: out = in0*in1*s0 - in0*s1  (i*g = 2*si*sg - si)."""
    global _IG_OP
    if _IG_OP is None:
        import re

        import concourse.dve_ops as dve_ops
        from concourse.dve_spec import C0, C1, Spec, Src0, Src1

        op = dve_ops.DveOp(
            "LSTM_IG_ANT",
            Spec(body=Src0 * Src1 * C0 - Src0 * C1,
                 reference=lambda in0, in1, s0, s1, imm2=0.0: in0 * in1 * s0 - in0 * s1),
            subdim=False, uops_sha={})
        dve_ops.OPS.append(op)
        dve_ops.CUSTOM_DVE_SPECS[op.name] = op.spec
        dve_ops._SUB_OPCODE_FOR_NAME[op.name] = (
            dve_ops._CUSTOM_DVE_ROW_BASE + len(dve_ops.OPS) - 1)
        for ver in ("v3", "v4"):
            try:
                op.compile(ver)
            except ValueError as e:
                m = re.search(r"v\d: ([0-9a-f]+) ", str(e))
                op.uops_sha[ver] = m.group(1)
                op.compile(ver)
        _IG_OP = op
    return _IG_OP


def emit_lstm(ctx, tc, io, S=S_FULL, n_chains=N_CHAINS):
    """Decoupled-chains LSTM recurrence.

    Each chain (B_C = B_LOC/n_chains batches) runs independently:
      MM zX(t) -> sigmoid(z) -> c-update (DVE) -> tanh(c) -> h (DVE) -> MM(t+1)
    Chains are emitted phase-offset so engines pipeline across chains.
    """
    nc = tc.nc
    C = n_chains
    B_C = B_LOC // C
    NW = S // T_W
    n_bp = B_C // 2
    GB = G * B_C               # gate cols per step slot
    assert S % T_W == 0 and B_C % 2 == 0

    x, whg, wxg, bg, wd, bd, yT = (io[k] for k in ("x", "whg", "wxg", "bg", "wd", "bd", "yT"))

    Xp = [nc.dram_tensor(f"Xp_{bl}", [S, 64], BF16).ap() for bl in range(B_LOC)]

    wpool = ctx.enter_context(tc.tile_pool(name="weights", bufs=1))
    spool = ctx.enter_context(tc.tile_pool(name="sig", bufs=12))
    dpool = ctx.enter_context(tc.tile_pool(name="dve", bufs=10))
    xrpool = ctx.enter_context(tc.tile_pool(name="xr", bufs=1))
    hpool = ctx.enter_context(tc.tile_pool(name="hwin", bufs=6))

    wh_sb = wpool.tile([128, 4 * H], BF16, tag="wh")
    nc.sync.dma_start(wh_sb[:], whg[:])
    wx_sb = wpool.tile([64, 4 * H], BF16, tag="wx")
    nc.sync.dma_start(wx_sb[:], wxg[:])
    bg_sb = wpool.tile([1, 4 * H], BF16, tag="bg")
    nc.sync.dma_start(bg_sb[:], bg[:])
    wd_sb = wpool.tile([128, 1], BF16, tag="wd")
    nc.sync.dma_start(wd_sb[:], wd[:])
    bd_sb = wpool.tile([1, 1], F32, tag="bd")
    nc.sync.dma_start(bd_sb[:], bd[:])
    ones_sb = wpool.tile([1, 512], BF16, tag="ones")
    nc.vector.memset(ones_sb[:], 1.0)
    zero_c = wpool.tile([128, B_C], F32, tag="zero_c")
    nc.vector.memset(zero_c[:], 0.0)

    # Cast x to bf16 DRAM scratch, then transpose once into resident SBUF
    # tiles [64, S] (one per batch) - no per-window transposes or shifts.
    for bl in range(B_LOC):
        nc.gpsimd.dma_start(Xp[bl][:, :], x[bl, :, :])
    xr = []
    XCH = min(512, S)
    for bl in range(B_LOC):
        t = xrpool.tile([64, S], BF16, tag=f"xr{bl}")
        for k in range(S // XCH):
            nc.sync.dma_start_transpose(
                t[:, k * XCH:(k + 1) * XCH], Xp[bl][k * XCH:(k + 1) * XCH, :])
        xr.append(t)

    zw_tiles = [dict() for _ in range(C)]
    hwin_tiles = {}
    s_cur = [None] * C
    c_state = [None] * C
    h_slices = {}

    with tc.tile_pool(name="psum", bufs=3, space="PSUM") as ppool:

        def t3(c, w):
            return zw_tiles[c][w][:].rearrange("p (r t) -> p r t", t=T_W)

        def prefill(w):
            for c in range(C):
                zw_tiles[c][w] = ppool.tile([128, 512], F32, tag=f"zw{c}", name=f"zw{c}_{w}")
            hwin_tiles[w] = hpool.tile([128, T_W * B_LOC], BF16, tag="hw", name=f"hw_{w}")

        def prefill_mms(w):
            for c in range(C):
                zw_t = zw_tiles[c][w]
                for g in range(G):
                    nc.tensor.matmul(
                        zw_t[:, g * B_C * T_W:(g + 1) * B_C * T_W],
                        bg_sb[0:1, H * g:H * (g + 1)],
                        ones_sb[0:1, 0:B_C * T_W],
                        start=True, stop=False, skip_group_check=True)
                    for b in range(B_C):
                        rhs = xr[c * B_C + b][:, w * T_W:(w + 1) * T_W]
                        nc.tensor.matmul(
                            zw_t[:, (g * B_C + b) * T_W:(g * B_C + b + 1) * T_W],
                            wx_sb[:, H * g:H * (g + 1)],
                            rhs,
                            start=False, stop=False, skip_group_check=True)

        def mm_z(c, t):
            w, tl = divmod(t, T_W)
            zt3 = t3(c, w)
            hsl = h_slices[(c, t - 1)]
            for g in range(G):
                nc.tensor.matmul(
                    zt3[:, g * B_C:(g + 1) * B_C, tl],
                    wh_sb[:, H * g:H * (g + 1)],
                    hsl,
                    start=False, stop=True, skip_group_check=True)

        def phase_a(c, t):
            # MMs + sigmoid over the 4 gates
            if t > 0:
                mm_z(c, t)
            w, tl = divmod(t, T_W)
            s = spool.tile([128, GB], F32, tag=f"s{c}", name=f"s{c}_t")
            nc.scalar.activation(s[:], t3(c, w)[:, 0:G * B_C, tl], AF.Sigmoid)
            s_cur[c] = s

        def phase_b(c, t):
            # c update, tanh, h
            w, tl = divmod(t, T_W)
            s = s_cur[c]
            si, sf, sg, so = (s[:, k * B_C:(k + 1) * B_C] for k in range(4))
            c_prev = zero_c[:] if t == 0 else c_state[c][:]
            wv = dpool.tile([128, B_C], F32, tag=f"w{c}", name=f"w{c}_t")
            nc.vector._custom_dve(get_ig_op(), out=wv[:], in0=si, in1=sg, s0=2.0, s1=1.0)
            v = dpool.tile([128, B_C], F32, tag=f"v{c}", name=f"v{c}_t")
            nc.vector.tensor_tensor(v[:], sf, c_prev, OP.mult)
            cn = dpool.tile([128, B_C], F32, tag=f"c{c}", name=f"c{c}_t", bufs=6)
            nc.vector.tensor_tensor(cn[:], v[:], wv[:], OP.add)
            c_state[c] = cn
            th = dpool.tile([128, B_C], F32, tag=f"th{c}", name=f"th{c}_t")
            nc.scalar.activation(th[:], cn[:], AF.Tanh)
            hsl = hwin_tiles[w][:, tl * B_LOC + c * B_C: tl * B_LOC + (c + 1) * B_C]
            nc.vector.tensor_tensor(hsl, so, th[:], OP.mult)
            h_slices[(c, t)] = hsl

        hc_pool = ctx.enter_context(tc.tile_pool(name="hc", bufs=2))
        WPC = 512 // (T_W * B_LOC)  # windows per 512-col head chunk

        def head_chunk(k):
            # y[0, 512k:512k+512] = sigmoid(Wd^T @ h_chunk + bd), h read
            # directly from the resident hwin SBUF tiles (no DRAM round trip)
            yp = ppool.tile([1, 512], F32, tag="yc", name=f"yc_{k}", bufs=1)
            ncol = T_W * B_LOC
            for j in range(WPC):
                nc.tensor.matmul(yp[:, ncol * j:ncol * (j + 1)],
                                 wd_sb[:], hwin_tiles[WPC * k + j][:],
                                 start=True, stop=True, skip_group_check=True)
            ys = hc_pool.tile([1, 512], F32, tag="ys", name=f"ys_{k}")
            nc.scalar.activation(ys[:], yp[:], AF.Sigmoid, bias=bd_sb[0:1, 0:1])
            nc.sync.dma_start(yT[0:1, 512 * k:512 * (k + 1)], ys[:])

        prefill(0)
        prefill_mms(0)

        for t in range(S):
            w, tl = divmod(t, T_W)
            if tl == 8 and w + 1 < NW:
                prefill(w + 1)
                prefill_mms(w + 1)
            if HEAD_INTERLEAVE and tl == 6 and w >= WPC and (w % WPC == 0):
                head_chunk(w // WPC - 1)

            if EMIT_ORDER == "ab_offset":
                # chain 0 phase A(t) | chains 1..: B(t-1) then A(t) | chain 0 B(t)
                phase_a(0, t)
                for c in range(1, C):
                    if t > 0:
                        phase_b(c, t - 1)
                    phase_a(c, t)
                phase_b(0, t)
            elif EMIT_ORDER == "seq":
                for c in range(C):
                    phase_a(c, t)
                    phase_b(c, t)
            elif EMIT_ORDER == "allA_allB":
                for c in range(C):
                    phase_a(c, t)
                for c in range(C):
                    phase_b(c, t)
            else:
                raise ValueError(EMIT_ORDER)
        if EMIT_ORDER == "ab_offset":
            for c in range(1, C):
                phase_b(c, S - 1)

        nchunks = S * B_LOC // 512
        done = len([w for w in range(WPC, NW) if w % WPC == 0]) if HEAD_INTERLEAVE else 0
        for k in range(done, nchunks):
            head_chunk(k)


def prep_weights(Wx, Wh, b, Wd, bd):
    """Host-side layout prep: fold tanh->sigmoid scale 2 into g-gate columns, cast bf16."""
    bf = ml_dtypes.bfloat16

    def scale_g(w):
        w = np.array(w, dtype=np.float32).copy()
        w[..., 2 * H:3 * H] *= 2.0
        return w.astype(bf)

    return dict(
        whg=scale_g(Wh),
        wxg=scale_g(Wx),
        bg=scale_g(np.asarray(b, np.float32).reshape(1, 4 * H)),
        wd=np.asarray(Wd, np.float32).astype(bf).reshape(H, 1),
        bd=np.asarray(bd, np.float32).reshape(1, 1),
        ident=np.eye(T_W, dtype=np.float32),
        bg4=scale_g(np.asarray(b, np.float32).reshape(1, 4 * H)).reshape(G, H),
        e4=np.kron(np.eye(G, dtype=np.float32),
                   np.ones((1, B_LOC), np.float32)).astype(bf),
    )


def strip_act_evsems(fn):
    """Merge [ACT EventSemaphore(w_x)] immediately followed by
    [ACT Activation(w_act_self)] into [ACT Activation(w_x)].

    The dropped wait is the bank-tracker's read-after-read ordering on the
    PSUM window tile: sigma(t) -> sigma(t-1) on the same in-order ACT engine,
    which is already implied transitively (sigma(t) <- PE matmul(t) <- ACT
    sigma(t-1) via the matmul's own bank-WAR wait).  Removing it keeps every
    instruction at <=1 wait so the ACT sequencer never blocks inside an
    EventSemaphore while later, ready work is queued behind it.
    """
    n = 0
    for bb in fn.blocks:
        insts = bb.instructions
        out = []
        k = 0
        while k < len(insts):
            i = insts[k]
            eng = str(i.engine).split(".")[-1]
            if (eng == "Activation" and i.opcode == "EventSemaphore"
                    and k + 1 < len(insts)):
                j = insts[k + 1]
                jeng = str(j.engine).split(".")[-1]
                iw = list(i.sync_info.on_wait) if i.sync_info else []
                jw = list(j.sync_info.on_wait) if j.sync_info else []
                iu = list(i.sync_info.on_update) if i.sync_info else []
                if (jeng == "Activation" and j.opcode == "Activation"
                        and len(iw) == 1 and not iu and len(jw) == 1
                        and "Activation_" in str(jw[0])):
                    j.sync_info.on_wait = [iw[0]]
                    out.append(j)
                    k += 2
                    n += 1
                    continue
            out.append(i)
            k += 1
        bb.instructions[:] = out
    return n


def strip_same_engine_waits(fn, engines=("DVE", "Activation"), margin=None):
    """Remove waits on an engine's OWN Tile semaphore when program order
    already implies them WITH SLACK.

    Tile emits one semaphore per engine (e.g. 'DVE_49'), incremented by that
    engine's instructions at completion.  In-order execution makes such a
    wait redundant — EXCEPT that engine writes retire up to ~init/2 cycles
    after the engine frees (the memory-ack window), so a read-after-write on
    the immediately preceding instruction still needs the semaphore.  We
    therefore only strip waits that are at least `margin` increments stale:
    the intervening instructions' engine-busy time covers the ack window
    (DVE: 2 x >=69ns > 129ns; ACT: 1 x >=190ns > 185ns).  WAR/RAR ordering
    is always safe in-order (reads complete during execution, before the
    next instruction starts).
    """
    margin = margin or {"DVE": 2, "Activation": 1}
    n = 0
    for bb in fn.blocks:
        counts = {}  # sem id -> inc count emitted so far by its own engine
        for i in bb.instructions:
            eng = str(i.engine).split(".")[-1]
            if not i.sync_info:
                continue
            if eng in engines:
                kept = []
                for w in i.sync_info.on_wait:
                    nm = getattr(w, "ant_name", "") or ""
                    if (nm.startswith(eng + "_")
                            and getattr(w, "wait_mode", "") == "sem-ge-imm"
                            and counts.get(nm, 0) >= (w.wait_value or 0) + margin[eng]):
                        n += 1
                        continue
                    kept.append(w)
                i.sync_info.on_wait = kept
            # track this engine's own-sem increments
            for u in (i.sync_info.on_update or []):
                nm = getattr(u, "ant_name", "") or ""
                if not nm.startswith(eng + "_"):
                    continue
                mode = getattr(u, "update_mode", "")
                if mode == "sem-inc":
                    counts[nm] = counts.get(nm, 0) + (u.update_value or 0)
                elif mode == "sem-wr-imm":
                    counts[nm] = u.update_value or 0
                else:
                    counts[nm] = -10**9  # unknown semantics: poison
    return n


def strip_transitive_pe_waits(fn):
    """Drop a DVE_x wait from an Activation-engine instruction when it is
    transitively implied by a PE_x wait on the same instruction: PE is
    in-order, so if an earlier PE instruction (e.g. the Ldweights preceding
    the gate matmuls) waits DVE_x >= a' with a' >= a, and this instruction
    waits PE_x >= b where the b-th PE increment comes from a later PE
    instruction, then PE_x >= b implies DVE_x >= a."""
    n = 0
    for bb in fn.blocks:
        pe_inc = 0
        dve_waits = []  # (pe_inc_before, dve_wait_value)
        for i in bb.instructions:
            eng = str(i.engine).split(".")[-1]
            if not i.sync_info:
                continue
            waits = list(i.sync_info.on_wait)
            if eng == "PE":
                for w in waits:
                    nm = getattr(w, "ant_name", "") or ""
                    if nm.startswith("DVE_") and getattr(w, "wait_mode", "") == "sem-ge-imm":
                        dve_waits.append((pe_inc, w.wait_value or 0))
                for u in (i.sync_info.on_update or []):
                    nm = getattr(u, "ant_name", "") or ""
                    if nm.startswith("PE_") and getattr(u, "update_mode", "") == "sem-inc":
                        pe_inc += u.update_value or 0
            elif eng == "Activation" and len(waits) >= 2:
                pe_w = [w for w in waits
                        if (getattr(w, "ant_name", "") or "").startswith("PE_")
                        and getattr(w, "wait_mode", "") == "sem-ge-imm"]
                if not pe_w:
                    continue
                b = min(w.wait_value or 0 for w in pe_w)
                kept = []
                for w in waits:
                    nm = getattr(w, "ant_name", "") or ""
                    if (nm.startswith("DVE_")
                            and getattr(w, "wait_mode", "") == "sem-ge-imm"
                            and any(cnt < b and a2 >= (w.wait_value or 0)
                                    for cnt, a2 in dve_waits)):
                        n += 1
                        continue
                    kept.append(w)
                i.sync_info.on_wait = kept
    return n


def fold_single_wait_evsems(fn, engines=("Activation", "DVE")):
    """[EvSem(<=1 wait, no updates)] [same-engine instr with no waits] ->
    move the wait onto the instruction and delete the EvSem.  EvSems with no
    waits and no updates are deleted outright."""
    n = 0
    for bb in fn.blocks:
        insts = bb.instructions
        out = []
        k = 0
        while k < len(insts):
            i = insts[k]
            eng = str(i.engine).split(".")[-1]
            if (eng in engines and i.opcode == "EventSemaphore" and i.sync_info
                    and not list(i.sync_info.on_update)
                    and len(list(i.sync_info.on_wait)) <= 1):
                waits = list(i.sync_info.on_wait)
                if not waits:
                    n += 1
                    k += 1
                    continue
                if k + 1 < len(insts):
                    j = insts[k + 1]
                    jeng = str(j.engine).split(".")[-1]
                    if (jeng == eng and j.opcode != "EventSemaphore"
                            and j.sync_info is not None
                            and not list(j.sync_info.on_wait)):
                        j.sync_info.on_wait = waits
                        out.append(j)
                        k += 2
                        n += 1
                        continue
            out.append(i)
            k += 1
        bb.instructions[:] = out
    return n


def _strided_ap(base_ap, part_stride, dims):
    """Return a copy of `base_ap` with hand-set free-dim strides.

    `dims` is [[stride, count], ...] in elements relative to the AP's base
    column.  Used for the interleaved reads of the fused cell update (pairs
    that straddle two regions of the same tile at a fixed column distance).
    """
    import bass_rust

    a = base_ap.copy()
    a.ap = bass_rust.VecI64Pair([[part_stride, 128]] + [list(d) for d in dims])
    return a


def emit_lstm_v2(ctx, tc, io, S=S_FULL):
    """Single-chain LSTM recurrence with a fused 2-op DVE cell update.

    Per step: 4 Wh matmuls (PSUM acc) -> ACT sigmoid over all four gates ->
    DVE IG (ig = 2*si*sg - si, i.e. i*tanh(g) with the g-gate pre-scaled by 2
    on the host) -> DVE tensor_tensor_scan computing c = f*c_prev + ig in one
    instruction -> ACT tanh -> DVE h = o*tanh(c) -> next MM.

    The scan streams interleaved pairs: state = (d0*state)+d1 over
    [0,f_j] x [c_prev_j, ig_j]; the 0-multiply resets the state at each batch
    boundary, so one 16-element instruction computes all 8 independent
    f*c_prev+ig updates.  All cell operands live in one persistent SBUF tile
    `cm` laid out so every scan operand is a CONTIGUOUS 2D slice:
      cols 0:64   sigmoid out at stride 2: i@0+2j, f@16+2j, g@32+2j, o@48+2j;
                  odd cols stay zero (memset once) = the scan reset zeros
      cols 64:81  scan out, even steps: [echo@64+2j | c@65+2j]; odd steps' IG
                  overwrites the echoes shifted by one: ig@66+2j
      cols 81:98  scan out, odd steps: [echo@81+2j | c@82+2j]; even steps'
                  ig@83+2j
    data0 = cm[15:31] = [0,f0,0,f1,...]; data1(p) = [c0,ig0,c1,ig1,...].
    """
    nc = tc.nc
    B_L = B_LOC
    GB = G * B_L
    NW = S // T_W
    assert S % T_W == 0 and GB * T_W == 512

    x, whg, wxg, bg, wd, bd, yT = (io[k] for k in ("x", "whg", "wxg", "bg", "wd", "bd", "yT"))

    wpool = ctx.enter_context(tc.tile_pool(name="weights", bufs=1))
    dpool = ctx.enter_context(tc.tile_pool(name="dve", bufs=8))
    xspool = ctx.enter_context(tc.tile_pool(name="xstg", bufs=2))
    xbpool = ctx.enter_context(tc.tile_pool(name="xbf", bufs=3))
    hpool = ctx.enter_context(tc.tile_pool(name="hwin", bufs=6))

    wh_sb = wpool.tile([128, 4 * H], BF16, tag="wh")
    nc.sync.dma_start(wh_sb[:], whg[:])
    wx_sb = wpool.tile([64, 4 * H], BF16, tag="wx")
    nc.sync.dma_start(wx_sb[:], wxg[:])
    b4_sb = wpool.tile([G, H], BF16, tag="b4")
    nc.sync.dma_start(b4_sb[:], io["bg4"][:])
    e4_sb = wpool.tile([G, G * B_L], BF16, tag="e4")
    nc.sync.dma_start(e4_sb[:], io["e4"][:])
    wd_sb = wpool.tile([128, 1], BF16, tag="wd")
    nc.sync.dma_start(wd_sb[:], wd[:])
    bd_sb = wpool.tile([1, 1], F32, tag="bd")
    nc.sync.dma_start(bd_sb[:], bd[:])
    ones_sb = wpool.tile([1, 512], BF16, tag="ones")
    nc.vector.memset(ones_sb[:], 1.0)
    zrow_sb = wpool.tile([1, 128], BF16, tag="zrow")
    nc.vector.memset(zrow_sb[:], 0.0)

    # persistent cell memory (layout in docstring)
    cm = wpool.tile([128, 100], F32, tag="cm")
    nc.vector.memset(cm[:, 0:100], 0.0)

    sig_out = cm[:, 0:64].rearrange("p (j e) -> p j e", e=2)[:, :, 0]  # stride-2
    si_ap = cm[:, 0:16].rearrange("p (j e) -> p j e", e=2)[:, :, 0]
    sg_ap = cm[:, 32:48].rearrange("p (j e) -> p j e", e=2)[:, :, 0]
    so_ap = cm[:, 48:64].rearrange("p (j e) -> p j e", e=2)[:, :, 0]
    d0 = cm[:, 15:31]                                   # [0,f0,0,f1,...]
    d1 = [cm[:, 82:98], cm[:, 65:81]]                   # [c_prev|ig] per parity
    ig_out = [cm[:, 83:99].rearrange("p (j e) -> p j e", e=2)[:, :, 0],
              cm[:, 66:82].rearrange("p (j e) -> p j e", e=2)[:, :, 0]]
    c_out = [cm[:, 64:80], cm[:, 81:97]]
    c_odd = [cm[:, 65:81].rearrange("p (j e) -> p j e", e=2)[:, :, 0],
             cm[:, 82:98].rearrange("p (j e) -> p j e", e=2)[:, :, 0]]

    # x windows flow in as ONE plain contiguous DMA per window (f32), are
    # transposed batch-by-batch on the tensor engine (identity-matmul into a
    # PSUM scratch bank) and cast to bf16 by a single DVE copy.  No
    # conversion DMAs, no xbar transposes; everything is off the critical
    # path and software-pipelined one window ahead.
    ident = io["ident"]
    id_sb = wpool.tile([T_W, T_W], F32, tag="ident")
    nc.sync.dma_start(id_sb[:], ident[:])
    xw_tiles = {}

    zw_tiles = {}
    hwin_tiles = {}

    with tc.tile_pool(name="psum", bufs=3, space="PSUM") as ppool:

        def t3(w):
            # t-major window layout: column = tl*GB + g*B_L + b, so every
            # per-step matmul output and the sigmoid input are CONTIGUOUS
            return zw_tiles[w][:].rearrange("p (t r) -> p t r", r=GB)

        def xprep_dma(w):
            xs = xspool.tile([T_W, B_L * D], F32, tag="xs", name=f"xs_{w}")
            nc.sync.dma_start(
                xs[:].rearrange("t (b d) -> t b d", d=D),
                x[:, w * T_W:(w + 1) * T_W, :].rearrange("b t d -> t b d"))
            xw_tiles[w] = [xs, None, None]

        def xprep_transpose(w):
            xs = xw_tiles[w][0]
            xtp = ppool.tile([64, B_L * T_W], F32, tag="xtp", name=f"xtp_{w}", bufs=2)
            for b in range(B_L):
                nc.tensor.transpose(xtp[:, T_W * b:T_W * (b + 1)],
                                    xs[:, D * b:D * (b + 1)], id_sb[:])
            xw_tiles[w][1] = xtp

        def xprep_cast(w):
            xbf = xbpool.tile([64, B_L * T_W], BF16, tag="xb", name=f"xb_{w}")
            nc.vector.tensor_copy(xbf[:], xw_tiles[w][1][:])
            xw_tiles[w][2] = xbf

        def prefill(w):
            zw_tiles[w] = ppool.tile([128, 512], F32, tag="zw", name=f"zw_{w}")
            hwin_tiles[w] = hpool.tile([128, T_W * B_L], BF16, tag="hw", name=f"hw_{w}")

        def prefill_zero(w):
            nc.tensor.matmul(zw_tiles[w][:, 0:512], zrow_sb[:], ones_sb[:],
                             start=True, stop=False, skip_group_check=True)

        def prefill_mms(w, ts):
            zt3 = t3(w)
            xbf3 = xw_tiles[w][2][:].rearrange("p (b t) -> p b t", t=T_W)
            for tp in ts:
                # bias for all 4 gates in one K=4 matmul: B4[g,:] x E4[g,col]
                nc.tensor.matmul(zt3[:, tp, :], b4_sb[:], e4_sb[:],
                                 start=False, stop=False, skip_group_check=True)
                for g in range(G):
                    nc.tensor.matmul(
                        zt3[:, tp, g * B_L:(g + 1) * B_L],
                        wx_sb[:, H * g:H * (g + 1)],
                        xbf3[:, :, tp],
                        start=False, stop=False, skip_group_check=True)

        hc_pool = ctx.enter_context(tc.tile_pool(name="hc", bufs=2))
        WPC = 512 // (T_W * B_L)

        def head_chunk(k):
            yp = ppool.tile([1, 512], F32, tag="yc", name=f"yc_{k}", bufs=1)
            ncol = T_W * B_L
            nc.tensor.matmul(yp[:], zrow_sb[0:1, 0:1], ones_sb[:],
                             start=True, stop=False, skip_group_check=True)
            for j in range(WPC):
                nc.tensor.matmul(yp[:, ncol * j:ncol * (j + 1)],
                                 wd_sb[:], hwin_tiles[WPC * k + j][:],
                                 start=False, stop=True, skip_group_check=True)
            ys = hc_pool.tile([1, 512], F32, tag="ys", name=f"ys_{k}")
            nc.scalar.activation(ys[:], yp[:], AF.Sigmoid, bias=bd_sb[0:1, 0:1])
            nc.sync.dma_start(yT[0:1, 512 * k:512 * (k + 1)], ys[:])

        xprep_dma(0)
        xprep_transpose(0)
        xprep_cast(0)
        prefill(0)
        prefill_zero(0)
        prefill_mms(0, ts=range(T_W))

        for t in range(S):
            w, tl = divmod(t, T_W)

            zt3 = t3(w)
            if t > 0:
                hsl_prev = hwin_tiles[divmod(t - 1, T_W)[0]][
                    :, divmod(t - 1, T_W)[1] * B_L:(divmod(t - 1, T_W)[1] + 1) * B_L]
                for g in range(G):
                    nc.tensor.matmul(
                        zt3[:, tl, g * B_L:(g + 1) * B_L],
                        wh_sb[:, H * g:H * (g + 1)],
                        hsl_prev,
                        start=False, stop=True, skip_group_check=True)
            # sigmoid over all four gates, written at stride 2 into cm
            nc.scalar.activation(sig_out, zt3[:, tl, 0:GB], AF.Sigmoid)
            p = t & 1
            # ig = 2*si*sg - si, interleaved into the previous scan-out region
            nc.vector._custom_dve(get_ig_op(), out=ig_out[p],
                                  in0=si_ap, in1=sg_ap, s0=2.0, s1=1.0)
            # c = f*c_prev + ig in one scan: state=(d0*state)+d1 over pairs
            nc.vector.tensor_tensor_scan(
                c_out[p], d0, d1[p], 0.0, OP.mult, OP.add)
            # tanh(c) then h = o*tanh(c) straight into the matmul rhs window
            th = dpool.tile([128, B_L], F32, tag="th", name=f"th_{t}")
            nc.scalar.activation(th[:], c_odd[p], AF.Tanh)
            hsl = hwin_tiles[w][:, tl * B_L:(tl + 1) * B_L]
            nc.vector.tensor_tensor(hsl, so_ap, th[:], OP.mult)

            # auxiliary work is emitted AFTER the step's recurrence ops so it
            # lands in each in-order engine queue behind the critical ops and
            # executes in their idle windows
            if w + 1 < NW:
                if tl == 0:
                    xprep_dma(w + 1)
                elif tl == 4:
                    xprep_transpose(w + 1)
                elif tl == 6:
                    xprep_cast(w + 1)
                elif tl == 7:
                    prefill(w + 1)
                    prefill_zero(w + 1)
                elif tl >= 8:
                    prefill_mms(w + 1, ts=[2 * (tl - 8), 2 * (tl - 8) + 1])
            if HEAD_INTERLEAVE and tl == 10 and w >= WPC and (w % WPC == 0):
                head_chunk(w // WPC - 1)

        nchunks = S * B_L // 512
        done = len([w for w in range(WPC, NW) if w % WPC == 0]) if HEAD_INTERLEAVE else 0
        for k in range(done, nchunks):
            head_chunk(k)


def build_nc(S=S_FULL, n_chains=N_CHAINS):
    nc = bacc.Bacc("TRN2", target_bir_lowering=False, debug=False)
    io = {
        "x": nc.dram_tensor("x", [B_LOC, S, D], F32, kind="ExternalInput").ap(),
        "whg": nc.dram_tensor("whg", [H, 4 * H], BF16, kind="ExternalInput").ap(),
        "wxg": nc.dram_tensor("wxg", [D, 4 * H], BF16, kind="ExternalInput").ap(),
        "bg": nc.dram_tensor("bg", [1, 4 * H], BF16, kind="ExternalInput").ap(),
        "wd": nc.dram_tensor("wd", [H, 1], BF16, kind="ExternalInput").ap(),
        "bd": nc.dram_tensor("bd", [1, 1], F32, kind="ExternalInput").ap(),
        "ident": nc.dram_tensor("ident", [T_W, T_W], F32, kind="ExternalInput").ap(),
        "bg4": nc.dram_tensor("bg4", [G, H], BF16, kind="ExternalInput").ap(),
        "e4": nc.dram_tensor("e4", [G, G * B_LOC], BF16, kind="ExternalInput").ap(),
        "yT": nc.dram_tensor("yT", [1, S * B_LOC], F32, kind="ExternalOutput").ap(),
    }
    with tile.TileContext(nc) as tc:
        with ExitStack() as ctx:
            if n_chains == 0:  # v2: fused-cell single chain
                emit_lstm_v2(ctx, tc, io, S=S)
            else:
                emit_lstm(ctx, tc, io, S=S, n_chains=n_chains)
    nc.compile()
    fn = nc.m.functions[0]
    strip_act_evsems(fn)
    strip_same_engine_waits(fn)
    strip_transitive_pe_waits(fn)
    fold_single_wait_evsems(fn)
    return nc


_CACHE = {}


def _get_compiled():
    if "nc" not in _CACHE:
        _CACHE["nc"] = build_nc()
    return _CACHE["nc"]


def _get_fast_runner():
    """Stable jitted SPMD callable, built once; subsequent kernel() calls only
    pay input transfer + dispatch + execute."""
    if "fast_run" in _CACHE:
        return _CACHE["fast_run"]

    import jax
    from jax.experimental.shard_map import shard_map
    from jax.sharding import Mesh, PartitionSpec

    from concourse import bass2jax

    nc = _get_compiled()
    bass2jax.install_neuronx_cc_hook()
    partition_name = nc.partition_id_tensor.name if nc.partition_id_tensor else None

    in_names, out_names, out_avals, zero_outs = [], [], [], []
    for alloc in nc.m.functions[0].allocations:
        if not isinstance(alloc, mybir.MemoryLocationSet):
            continue
        name = alloc.memorylocations[0].name
        if alloc.kind == "ExternalInput":
            if name != partition_name:
                in_names.append(name)
        elif alloc.kind == "ExternalOutput":
            out_names.append(name)
            shape = tuple(alloc.tensor_shape)
            dtype = mybir.dt.np(alloc.dtype)
            out_avals.append(jax.core.ShapedArray(shape, dtype))
            zero_outs.append(np.zeros((NCORES * shape[0], *shape[1:]), dtype))
    n_params = len(in_names)
    all_names = in_names + out_names
    if partition_name is not None:
        all_names = all_names + [partition_name]

    def _body(*args):
        operands = list(args)
        if partition_name is not None:
            operands.append(bass2jax.partition_id_tensor())
        outs = bass2jax._bass_exec_p.bind(
            *operands,
            out_avals=tuple(out_avals),
            in_names=tuple(all_names),
            out_names=tuple(out_names),
            lowering_input_output_aliases=(),
            sim_require_finite=True,
            sim_require_nnan=True,
            nc=nc,
        )
        return tuple(outs)

    devices = jax.devices()[:NCORES]
    mesh = Mesh(np.asarray(devices), ("core",))
    donate = tuple(range(n_params, n_params + len(out_names)))
    sharded = jax.jit(
        shard_map(_body, mesh=mesh,
                  in_specs=(PartitionSpec("core"),) * (n_params + len(out_names)),
                  out_specs=(PartitionSpec("core"),) * len(out_names),
                  check_rep=False),
        donate_argnums=donate, keep_unused=True)

    def run(concat_by_name):
        ins = [concat_by_name[k] for k in in_names]
        zeros = [z.copy() for z in zero_outs]  # donated each call
        out_arrs = sharded(*ins, *zeros)
        return {k: np.asarray(out_arrs[i]) for i, k in enumerate(out_names)}

    _CACHE["fast_run"] = run
    return run


def kernel(**inputs):
    x = np.ascontiguousarray(np.asarray(inputs["x"], np.float32))
    w = prep_weights(inputs["Wx"], inputs["Wh"], inputs["b"], inputs["Wd"], inputs["bd"])
    run = _get_fast_runner()
    # shard_map splits axis 0 across the 8 cores: per-core x shard
    # [B_LOC, S, D] concatenated along axis 0 is exactly the full x.
    concat = {"x": x}
    for k, v in w.items():
        concat[k] = np.concatenate([v] * NCORES, axis=0)
    outs = run(concat)
    yt = outs["yT"].reshape(NCORES, S_FULL, B_LOC)
    y = np.zeros((B, S_FULL, 1), np.float32)
    for c in range(NCORES):
        y[c * B_LOC:(c + 1) * B_LOC, :, 0] = yt[c].T
    return y


# ---------------------------------------------------------------------------
# Stable-jit SPMD runner (mirrors bass_utils.run_bass_kernel_spmd's axon path
# but keeps one jitted callable so repeated runs don't recompile).

def make_runner(nc, n_cores=NCORES):
    import jax
    from jax.experimental.shard_map import shard_map
    from jax.sharding import Mesh, PartitionSpec

    from concourse import bass2jax

    bass2jax.install_neuronx_cc_hook()
    assert nc.dbg_addr is None
    partition_name = nc.partition_id_tensor.name if nc.partition_id_tensor else None

    in_names, out_names, out_avals, zero_outs = [], [], [], []
    for alloc in nc.m.functions[0].allocations:
        if not isinstance(alloc, mybir.MemoryLocationSet):
            continue
        name = alloc.memorylocations[0].name
        if alloc.kind == "ExternalInput":
            if name != partition_name:
                in_names.append(name)
        elif alloc.kind == "ExternalOutput":
            out_names.append(name)
            shape = tuple(alloc.tensor_shape)
            dtype = mybir.dt.np(alloc.dtype)
            out_avals.append(jax.core.ShapedArray(shape, dtype))
            zero_outs.append(np.zeros(shape, dtype))
    n_params = len(in_names)
    all_names = in_names + out_names
    if partition_name is not None:
        all_names = all_names + [partition_name]

    def _body(*args):
        operands = list(args)
        if partition_name is not None:
            operands.append(bass2jax.partition_id_tensor())
        outs = bass2jax._bass_exec_p.bind(
            *operands,
            out_avals=tuple(out_avals),
            in_names=tuple(all_names),
            out_names=tuple(out_names),
            lowering_input_output_aliases=(),
            sim_require_finite=True,
            sim_require_nnan=True,
            nc=nc,
        )
        return tuple(outs)

    devices = jax.devices()[:n_cores]
    mesh = Mesh(np.asarray(devices), ("core",))
    donate = tuple(range(n_params, n_params + len(out_names)))
    sharded = jax.jit(
        shard_map(_body, mesh=mesh,
                  in_specs=(PartitionSpec("core"),) * (n_params + len(out_names)),
                  out_specs=(PartitionSpec("core"),) * len(out_names),
                  check_rep=False),
        donate_argnums=donate, keep_unused=True)

    def run(in_maps):
        concat_in = [np.concatenate([np.asarray(in_maps[c][k]) for c in range(n_cores)], axis=0)
                     for k in in_names]
        concat_zero = [np.zeros((n_cores * z.shape[0], *z.shape[1:]), z.dtype) for z in zero_outs]
        out_arrs = sharded(*concat_in, *concat_zero)
        return [
            {k: np.asarray(out_arrs[i]).reshape(n_cores, *out_avals[i].shape)[c]
             for i, k in enumerate(out_names)}
            for c in range(n_cores)
        ]

    return run


def make_null_nc(S=S_FULL):
    """Same external IO signature as the LSTM kernel, but only a token DMA —
    for calibrating per-call dispatch overhead in timing runs."""
    nc = bacc.Bacc("TRN2", target_bir_lowering=False, debug=False)
    x = nc.dram_tensor("x", [B_LOC, S, D], F32, kind="ExternalInput").ap()
    nc.dram_tensor("whg", [H, 4 * H], BF16, kind="ExternalInput").ap()
    nc.dram_tensor("wxg", [D, 4 * H], BF16, kind="ExternalInput").ap()
    nc.dram_tensor("bg", [1, 4 * H], BF16, kind="ExternalInput").ap()
    nc.dram_tensor("wd", [H, 1], BF16, kind="ExternalInput").ap()
    nc.dram_tensor("bd", [1, 1], F32, kind="ExternalInput").ap()
    yT = nc.dram_tensor("yT", [1, S * B_LOC], F32, kind="ExternalOutput").ap()
    with tile.TileContext(nc) as tc:
        with tc.tile_pool(name="p", bufs=1) as p:
            t = p.tile([1, 512], F32, name="tnull")
            nc.sync.dma_start(t[:], x[0, 0:8, 0:64].rearrange("a b -> (a b)")[None, :])
            nc.sync.dma_start(yT[0:1, 0:512], t[:])
    nc.compile()
    return nc


def make_device_runner(nc, n_cores=NCORES, n_zero_sets=12):
    """Like make_runner but with inputs pre-placed on device; returns
    (prepare(in_maps) -> None, run_once() -> outs) for tight timing loops."""
    import jax
    from jax.experimental.shard_map import shard_map
    from jax.sharding import Mesh, NamedSharding, PartitionSpec

    from concourse import bass2jax

    bass2jax.install_neuronx_cc_hook()
    partition_name = nc.partition_id_tensor.name if nc.partition_id_tensor else None
    in_names, out_names, out_avals, zero_outs = [], [], [], []
    for alloc in nc.m.functions[0].allocations:
        if not isinstance(alloc, mybir.MemoryLocationSet):
            continue
        name = alloc.memorylocations[0].name
        if alloc.kind == "ExternalInput":
            if name != partition_name:
                in_names.append(name)
        elif alloc.kind == "ExternalOutput":
            out_names.append(name)
            shape = tuple(alloc.tensor_shape)
            dtype = mybir.dt.np(alloc.dtype)
            out_avals.append(jax.core.ShapedArray(shape, dtype))
            zero_outs.append(np.zeros(shape, dtype))
    n_params = len(in_names)
    all_names = in_names + out_names
    if partition_name is not None:
        all_names = all_names + [partition_name]

    def _body(*args):
        operands = list(args)
        if partition_name is not None:
            operands.append(bass2jax.partition_id_tensor())
        outs = bass2jax._bass_exec_p.bind(
            *operands,
            out_avals=tuple(out_avals),
            in_names=tuple(all_names),
            out_names=tuple(out_names),
            lowering_input_output_aliases=(),
            sim_require_finite=True,
            sim_require_nnan=True,
            nc=nc,
        )
        return tuple(outs)

    devices = jax.devices()[:n_cores]
    mesh = Mesh(np.asarray(devices), ("core",))
    donate = tuple(range(n_params, n_params + len(out_names)))
    sharded = jax.jit(
        shard_map(_body, mesh=mesh,
                  in_specs=(PartitionSpec("core"),) * (n_params + len(out_names)),
                  out_specs=(PartitionSpec("core"),) * len(out_names),
                  check_rep=False),
        donate_argnums=donate, keep_unused=True)
    shard = NamedSharding(mesh, PartitionSpec("core"))

    state = {}

    def prepare(in_maps):
        concat_in = [np.concatenate([np.asarray(in_maps[c][k]) for c in range(n_cores)], axis=0)
                     for k in in_names]
        state["dev_in"] = [jax.device_put(a, shard) for a in concat_in]
        state["zeros"] = [
            [jax.device_put(np.zeros((n_cores * z.shape[0], *z.shape[1:]), z.dtype), shard)
             for z in zero_outs]
            for _ in range(n_zero_sets)
        ]
        state["k"] = 0

    def run_once():
        zs = state["zeros"][state["k"] % len(state["zeros"])]
        state["k"] += 1
        out = sharded(*state["dev_in"], *zs)
        jax.block_until_ready(out)
        return out

    return prepare, run_once



# revision 52
# speedup vs baseline: 1.0113x; 1.0113x over previous
"""Trainium2 Bass kernel for nn_BaseLSTM: y = sigmoid(Dense(LSTM(x))).

Reference (per batch b, time t):
    xz = x @ Wx + b                      # [B,S,4H], keras gate order i,f,g,o
    z_t = xz_t + h_{t-1} @ Wh
    i,f,o = sigmoid(z_i), sigmoid(z_f), sigmoid(z_o);  g = tanh(z_g)
    c_t = f*c + i*g;  h_t = o * tanh(c_t)
    y_t = sigmoid(h_t @ Wd + bd)

Sharding: data-parallel over batch, 8 batches per core on 8 cores.

Per-core design (B_LOC=8 batches, split into pairs of coupled "chains"):
  - All matmuls in bf16 (fp32 PSUM accumulation).
  - tanh is computed as 2*sigmoid(2x)-1 so every transcendental is a sigmoid;
    the g-gate pre-activation is pre-scaled by 2 by folding the factor into
    Wx/Wh/b g-columns on the host.  The cell state is kept as c2 = 2*c so
    tanh(c) = 2*sigmoid(c2)-1 with no extra scaling op.
  - PSUM "window" layout per chain: bank tile [128, T_W*5*B_C] f32, per step
    slot [z_i | z_f | z_g | z_o | c2] (each B_C cols).  A window is pre-filled
    by PE matmuls: bias (K=1, rhs=ones) then Wx (K=64, rhs = transposed x
    window); the per-step Wh matmuls (K=128, rhs=h) accumulate on top.  The
    sigmoid over one slot [128, 5*B_C] then yields all four gates AND the
    partner chain's tanh argument in a single ACT instruction.
  - x is transposed via the DMA XBAR: x (f32, [S,64] per batch) is cast to
    bf16 into DRAM scratch Xp [S, 128] (two batches side by side), then
    dma_start_transpose lifts [T_W,128] -> [128,T_W] SBUF tiles per window.
  - h_t (bf16) is written by the DVE straight into a per-window SBUF tile
    that also serves as the next step's matmul rhs; windows are DMA'd to DRAM
    and re-read for the final Dense+sigmoid pass (PE K=128 M=1 matmul).
"""

import time
from contextlib import ExitStack

import ml_dtypes
import numpy as np

import concourse.bacc as bacc
import concourse.mybir as mybir
import concourse.tile as tile
from concourse import bass_utils

F32 = mybir.dt.float32
BF16 = mybir.dt.bfloat16
AF = mybir.ActivationFunctionType
OP = mybir.AluOpType

B, S_FULL, D, H = 64, 2048, 64, 128
NCORES = 8
B_LOC = B // NCORES  # 8
G = 4
T_W = 16             # steps per PSUM window
N_CHAINS = 0         # 0 = v2 fused-cell single chain; >=1 = legacy chains
CELL_MODE = "mul3"   # 'scan' = tensor_tensor_scan cell; 'mul3' = 3-op cell
EMIT_ORDER = "ab_offset"  # emission interleave of per-chain phases
HEAD_INTERLEAVE = True   # fold dense-head chunks into the recurrence stream


_IG_OP = None


def get_ig_op():
    """Custom DVE op: out = in0*in1*s0 - in0*s1  (i*g = 2*si*sg - si)."""
    global _IG_OP
    if _IG_OP is None:
        import re

        import concourse.dve_ops as dve_ops
        from concourse.dve_spec import C0, C1, Spec, Src0, Src1

        op = dve_ops.DveOp(
            "LSTM_IG_ANT",
            Spec(body=Src0 * Src1 * C0 - Src0 * C1,
                 reference=lambda in0, in1, s0, s1, imm2=0.0: in0 * in1 * s0 - in0 * s1),
            subdim=False, uops_sha={})
        dve_ops.OPS.append(op)
        dve_ops.CUSTOM_DVE_SPECS[op.name] = op.spec
        dve_ops._SUB_OPCODE_FOR_NAME[op.name] = (
            dve_ops._CUSTOM_DVE_ROW_BASE + len(dve_ops.OPS) - 1)
        for ver in ("v3", "v4"):
            try:
                op.compile(ver)
            except ValueError as e:
                m = re.search(r"v\d: ([0-9a-f]+) ", str(e))
                op.uops_sha[ver] = m.group(1)
                op.compile(ver)
        _IG_OP = op
    return _IG_OP


def emit_lstm(ctx, tc, io, S=S_FULL, n_chains=N_CHAINS):
    """Decoupled-chains LSTM recurrence.

    Each chain (B_C = B_LOC/n_chains batches) runs independently:
      MM zX(t) -> sigmoid(z) -> c-update (DVE) -> tanh(c) -> h (DVE) -> MM(t+1)
    Chains are emitted phase-offset so engines pipeline across chains.
    """
    nc = tc.nc
    C = n_chains
    B_C = B_LOC // C
    NW = S // T_W
    n_bp = B_C // 2
    GB = G * B_C               # gate cols per step slot
    assert S % T_W == 0 and B_C % 2 == 0

    x, whg, wxg, bg, wd, bd, yT = (io[k] for k in ("x", "whg", "wxg", "bg", "wd", "bd", "yT"))

    Xp = [nc.dram_tensor(f"Xp_{bl}", [S, 64], BF16).ap() for bl in range(B_LOC)]

    wpool = ctx.enter_context(tc.tile_pool(name="weights", bufs=1))
    spool = ctx.enter_context(tc.tile_pool(name="sig", bufs=12))
    dpool = ctx.enter_context(tc.tile_pool(name="dve", bufs=10))
    xrpool = ctx.enter_context(tc.tile_pool(name="xr", bufs=1))
    hpool = ctx.enter_context(tc.tile_pool(name="hwin", bufs=6))

    wh_sb = wpool.tile([128, 4 * H], BF16, tag="wh")
    nc.sync.dma_start(wh_sb[:], whg[:])
    wx_sb = wpool.tile([64, 4 * H], BF16, tag="wx")
    nc.sync.dma_start(wx_sb[:], wxg[:])
    bg_sb = wpool.tile([1, 4 * H], BF16, tag="bg")
    nc.sync.dma_start(bg_sb[:], bg[:])
    wd_sb = wpool.tile([128, 1], BF16, tag="wd")
    nc.sync.dma_start(wd_sb[:], wd[:])
    bd_sb = wpool.tile([1, 1], F32, tag="bd")
    nc.sync.dma_start(bd_sb[:], bd[:])
    ones_sb = wpool.tile([1, 512], BF16, tag="ones")
    nc.vector.memset(ones_sb[:], 1.0)
    zero_c = wpool.tile([128, B_C], F32, tag="zero_c")
    nc.vector.memset(zero_c[:], 0.0)

    # Cast x to bf16 DRAM scratch, then transpose once into resident SBUF
    # tiles [64, S] (one per batch) - no per-window transposes or shifts.
    for bl in range(B_LOC):
        nc.gpsimd.dma_start(Xp[bl][:, :], x[bl, :, :])
    xr = []
    XCH = min(512, S)
    for bl in range(B_LOC):
        t = xrpool.tile([64, S], BF16, tag=f"xr{bl}")
        for k in range(S // XCH):
            nc.sync.dma_start_transpose(
                t[:, k * XCH:(k + 1) * XCH], Xp[bl][k * XCH:(k + 1) * XCH, :])
        xr.append(t)

    zw_tiles = [dict() for _ in range(C)]
    hwin_tiles = {}
    s_cur = [None] * C
    c_state = [None] * C
    h_slices = {}

    with tc.tile_pool(name="psum", bufs=3, space="PSUM") as ppool:

        def t3(c, w):
            return zw_tiles[c][w][:].rearrange("p (r t) -> p r t", t=T_W)

        def prefill(w):
            for c in range(C):
                zw_tiles[c][w] = ppool.tile([128, 512], F32, tag=f"zw{c}", name=f"zw{c}_{w}")
            hwin_tiles[w] = hpool.tile([128, T_W * B_LOC], BF16, tag="hw", name=f"hw_{w}")

        def prefill_mms(w):
            for c in range(C):
                zw_t = zw_tiles[c][w]
                for g in range(G):
                    nc.tensor.matmul(
                        zw_t[:, g * B_C * T_W:(g + 1) * B_C * T_W],
                        bg_sb[0:1, H * g:H * (g + 1)],
                        ones_sb[0:1, 0:B_C * T_W],
                        start=True, stop=False, skip_group_check=True)
                    for b in range(B_C):
                        rhs = xr[c * B_C + b][:, w * T_W:(w + 1) * T_W]
                        nc.tensor.matmul(
                            zw_t[:, (g * B_C + b) * T_W:(g * B_C + b + 1) * T_W],
                            wx_sb[:, H * g:H * (g + 1)],
                            rhs,
                            start=False, stop=False, skip_group_check=True)

        def mm_z(c, t):
            w, tl = divmod(t, T_W)
            zt3 = t3(c, w)
            hsl = h_slices[(c, t - 1)]
            for g in range(G):
                nc.tensor.matmul(
                    zt3[:, g * B_C:(g + 1) * B_C, tl],
                    wh_sb[:, H * g:H * (g + 1)],
                    hsl,
                    start=False, stop=True, skip_group_check=True)

        def phase_a(c, t):
            # MMs + sigmoid over the 4 gates
            if t > 0:
                mm_z(c, t)
            w, tl = divmod(t, T_W)
            s = spool.tile([128, GB], F32, tag=f"s{c}", name=f"s{c}_t")
            nc.scalar.activation(s[:], t3(c, w)[:, 0:G * B_C, tl], AF.Sigmoid)
            s_cur[c] = s

        def phase_b(c, t):
            # c update, tanh, h
            w, tl = divmod(t, T_W)
            s = s_cur[c]
            si, sf, sg, so = (s[:, k * B_C:(k + 1) * B_C] for k in range(4))
            c_prev = zero_c[:] if t == 0 else c_state[c][:]
            wv = dpool.tile([128, B_C], F32, tag=f"w{c}", name=f"w{c}_t")
            nc.vector._custom_dve(get_ig_op(), out=wv[:], in0=si, in1=sg, s0=2.0, s1=1.0)
            v = dpool.tile([128, B_C], F32, tag=f"v{c}", name=f"v{c}_t")
            nc.vector.tensor_tensor(v[:], sf, c_prev, OP.mult)
            cn = dpool.tile([128, B_C], F32, tag=f"c{c}", name=f"c{c}_t", bufs=6)
            nc.vector.tensor_tensor(cn[:], v[:], wv[:], OP.add)
            c_state[c] = cn
            th = dpool.tile([128, B_C], F32, tag=f"th{c}", name=f"th{c}_t")
            nc.scalar.activation(th[:], cn[:], AF.Tanh)
            hsl = hwin_tiles[w][:, tl * B_LOC + c * B_C: tl * B_LOC + (c + 1) * B_C]
            nc.vector.tensor_tensor(hsl, so, th[:], OP.mult)
            h_slices[(c, t)] = hsl

        hc_pool = ctx.enter_context(tc.tile_pool(name="hc", bufs=2))
        WPC = 512 // (T_W * B_LOC)  # windows per 512-col head chunk

        def head_chunk(k):
            # y[0, 512k:512k+512] = sigmoid(Wd^T @ h_chunk + bd), h read
            # directly from the resident hwin SBUF tiles (no DRAM round trip)
            yp = ppool.tile([1, 512], F32, tag="yc", name=f"yc_{k}", bufs=1)
            ncol = T_W * B_LOC
            for j in range(WPC):
                nc.tensor.matmul(yp[:, ncol * j:ncol * (j + 1)],
                                 wd_sb[:], hwin_tiles[WPC * k + j][:],
                                 start=True, stop=True, skip_group_check=True)
            ys = hc_pool.tile([1, 512], F32, tag="ys", name=f"ys_{k}")
            nc.scalar.activation(ys[:], yp[:], AF.Sigmoid, bias=bd_sb[0:1, 0:1])
            nc.sync.dma_start(yT[0:1, 512 * k:512 * (k + 1)], ys[:])

        prefill(0)
        prefill_mms(0)

        for t in range(S):
            w, tl = divmod(t, T_W)
            if tl == 8 and w + 1 < NW:
                prefill(w + 1)
                prefill_mms(w + 1)
            if HEAD_INTERLEAVE and tl == 6 and w >= WPC and (w % WPC == 0):
                head_chunk(w // WPC - 1)

            if EMIT_ORDER == "ab_offset":
                # chain 0 phase A(t) | chains 1..: B(t-1) then A(t) | chain 0 B(t)
                phase_a(0, t)
                for c in range(1, C):
                    if t > 0:
                        phase_b(c, t - 1)
                    phase_a(c, t)
                phase_b(0, t)
            elif EMIT_ORDER == "seq":
                for c in range(C):
                    phase_a(c, t)
                    phase_b(c, t)
            elif EMIT_ORDER == "allA_allB":
                for c in range(C):
                    phase_a(c, t)
                for c in range(C):
                    phase_b(c, t)
            else:
                raise ValueError(EMIT_ORDER)
        if EMIT_ORDER == "ab_offset":
            for c in range(1, C):
                phase_b(c, S - 1)

        nchunks = S * B_LOC // 512
        done = len([w for w in range(WPC, NW) if w % WPC == 0]) if HEAD_INTERLEAVE else 0
        for k in range(done, nchunks):
            head_chunk(k)


def prep_weights(Wx, Wh, b, Wd, bd):
    """Host-side layout prep: fold tanh->sigmoid scale 2 into g-gate columns, cast bf16."""
    bf = ml_dtypes.bfloat16

    def scale_g(w):
        w = np.array(w, dtype=np.float32).copy()
        w[..., 2 * H:3 * H] *= 2.0
        return w.astype(bf)

    return dict(
        whg=scale_g(Wh),
        wxg=scale_g(Wx),
        bg=scale_g(np.asarray(b, np.float32).reshape(1, 4 * H)),
        wd=np.asarray(Wd, np.float32).astype(bf).reshape(H, 1),
        bd=np.asarray(bd, np.float32).reshape(1, 1),
        ident=np.eye(T_W, dtype=np.float32),
        bg4=scale_g(np.asarray(b, np.float32).reshape(1, 4 * H)).reshape(G, H),
        e4=np.kron(np.eye(G, dtype=np.float32),
                   np.ones((1, B_LOC), np.float32)).astype(bf),
    )


def strip_act_evsems(fn):
    """Merge [ACT EventSemaphore(w_x)] immediately followed by
    [ACT Activation(w_act_self)] into [ACT Activation(w_x)].

    The dropped wait is the bank-tracker's read-after-read ordering on the
    PSUM window tile: sigma(t) -> sigma(t-1) on the same in-order ACT engine,
    which is already implied transitively (sigma(t) <- PE matmul(t) <- ACT
    sigma(t-1) via the matmul's own bank-WAR wait).  Removing it keeps every
    instruction at <=1 wait so the ACT sequencer never blocks inside an
    EventSemaphore while later, ready work is queued behind it.
    """
    n = 0
    for bb in fn.blocks:
        insts = bb.instructions
        out = []
        k = 0
        while k < len(insts):
            i = insts[k]
            eng = str(i.engine).split(".")[-1]
            if (eng == "Activation" and i.opcode == "EventSemaphore"
                    and k + 1 < len(insts)):
                j = insts[k + 1]
                jeng = str(j.engine).split(".")[-1]
                iw = list(i.sync_info.on_wait) if i.sync_info else []
                jw = list(j.sync_info.on_wait) if j.sync_info else []
                iu = list(i.sync_info.on_update) if i.sync_info else []
                if (jeng == "Activation" and j.opcode == "Activation"
                        and len(iw) == 1 and not iu and len(jw) == 1
                        and "Activation_" in str(jw[0])):
                    j.sync_info.on_wait = [iw[0]]
                    out.append(j)
                    k += 2
                    n += 1
                    continue
            out.append(i)
            k += 1
        bb.instructions[:] = out
    return n


def strip_same_engine_waits(fn, engines=("DVE", "Activation"), margin=None):
    """Remove waits on an engine's OWN Tile semaphore when program order
    already implies them WITH SLACK.

    Tile emits one semaphore per engine (e.g. 'DVE_49'), incremented by that
    engine's instructions at completion.  In-order execution makes such a
    wait redundant — EXCEPT that engine writes retire up to ~init/2 cycles
    after the engine frees (the memory-ack window), so a read-after-write on
    the immediately preceding instruction still needs the semaphore.  We
    therefore only strip waits that are at least `margin` increments stale:
    the intervening instructions' engine-busy time covers the ack window
    (DVE: 2 x >=69ns > 129ns; ACT: 1 x >=190ns > 185ns).  WAR/RAR ordering
    is always safe in-order (reads complete during execution, before the
    next instruction starts).
    """
    margin = margin or {"DVE": 2, "Activation": 1}
    n = 0
    for bb in fn.blocks:
        counts = {}  # sem id -> inc count emitted so far by its own engine
        for i in bb.instructions:
            eng = str(i.engine).split(".")[-1]
            if not i.sync_info:
                continue
            if eng in engines:
                kept = []
                for w in i.sync_info.on_wait:
                    nm = getattr(w, "ant_name", "") or ""
                    if (nm.startswith(eng + "_")
                            and getattr(w, "wait_mode", "") == "sem-ge-imm"
                            and counts.get(nm, 0) >= (w.wait_value or 0) + margin[eng]):
                        n += 1
                        continue
                    kept.append(w)
                i.sync_info.on_wait = kept
            # track this engine's own-sem increments
            for u in (i.sync_info.on_update or []):
                nm = getattr(u, "ant_name", "") or ""
                if not nm.startswith(eng + "_"):
                    continue
                mode = getattr(u, "update_mode", "")
                if mode == "sem-inc":
                    counts[nm] = counts.get(nm, 0) + (u.update_value or 0)
                elif mode == "sem-wr-imm":
                    counts[nm] = u.update_value or 0
                else:
                    counts[nm] = -10**9  # unknown semantics: poison
    return n


def strip_transitive_pe_waits(fn):
    """Drop a DVE_x wait from an Activation-engine instruction when it is
    transitively implied by a PE_x wait on the same instruction: PE is
    in-order, so if an earlier PE instruction (e.g. the Ldweights preceding
    the gate matmuls) waits DVE_x >= a' with a' >= a, and this instruction
    waits PE_x >= b where the b-th PE increment comes from a later PE
    instruction, then PE_x >= b implies DVE_x >= a."""
    n = 0
    for bb in fn.blocks:
        pe_inc = 0
        dve_waits = []  # (pe_inc_before, dve_wait_value)
        for i in bb.instructions:
            eng = str(i.engine).split(".")[-1]
            if not i.sync_info:
                continue
            waits = list(i.sync_info.on_wait)
            if eng == "PE":
                for w in waits:
                    nm = getattr(w, "ant_name", "") or ""
                    if nm.startswith("DVE_") and getattr(w, "wait_mode", "") == "sem-ge-imm":
                        dve_waits.append((pe_inc, w.wait_value or 0))
                for u in (i.sync_info.on_update or []):
                    nm = getattr(u, "ant_name", "") or ""
                    if nm.startswith("PE_") and getattr(u, "update_mode", "") == "sem-inc":
                        pe_inc += u.update_value or 0
            elif eng == "Activation" and len(waits) >= 2:
                pe_w = [w for w in waits
                        if (getattr(w, "ant_name", "") or "").startswith("PE_")
                        and getattr(w, "wait_mode", "") == "sem-ge-imm"]
                if not pe_w:
                    continue
                b = min(w.wait_value or 0 for w in pe_w)
                kept = []
                for w in waits:
                    nm = getattr(w, "ant_name", "") or ""
                    if (nm.startswith("DVE_")
                            and getattr(w, "wait_mode", "") == "sem-ge-imm"
                            and any(cnt < b and a2 >= (w.wait_value or 0)
                                    for cnt, a2 in dve_waits)):
                        n += 1
                        continue
                    kept.append(w)
                i.sync_info.on_wait = kept
    return n


def fold_single_wait_evsems(fn, engines=("Activation", "DVE")):
    """[EvSem(<=1 wait, no updates)] [same-engine instr with no waits] ->
    move the wait onto the instruction and delete the EvSem.  EvSems with no
    waits and no updates are deleted outright."""
    n = 0
    for bb in fn.blocks:
        insts = bb.instructions
        out = []
        k = 0
        while k < len(insts):
            i = insts[k]
            eng = str(i.engine).split(".")[-1]
            if (eng in engines and i.opcode == "EventSemaphore" and i.sync_info
                    and not list(i.sync_info.on_update)
                    and len(list(i.sync_info.on_wait)) <= 1):
                waits = list(i.sync_info.on_wait)
                if not waits:
                    n += 1
                    k += 1
                    continue
                if k + 1 < len(insts):
                    j = insts[k + 1]
                    jeng = str(j.engine).split(".")[-1]
                    if (jeng == eng and j.opcode != "EventSemaphore"
                            and j.sync_info is not None
                            and not list(j.sync_info.on_wait)):
                        j.sync_info.on_wait = waits
                        out.append(j)
                        k += 2
                        n += 1
                        continue
            out.append(i)
            k += 1
        bb.instructions[:] = out
    return n


def _strided_ap(base_ap, part_stride, dims):
    """Return a copy of `base_ap` with hand-set free-dim strides.

    `dims` is [[stride, count], ...] in elements relative to the AP's base
    column.  Used for the interleaved reads of the fused cell update (pairs
    that straddle two regions of the same tile at a fixed column distance).
    """
    import bass_rust

    a = base_ap.copy()
    a.ap = bass_rust.VecI64Pair([[part_stride, 128]] + [list(d) for d in dims])
    return a


def emit_lstm_v2(ctx, tc, io, S=S_FULL):
    """Single-chain LSTM recurrence with a fused 2-op DVE cell update.

    Per step: 4 Wh matmuls (PSUM acc) -> ACT sigmoid over all four gates ->
    DVE IG (ig = 2*si*sg - si, i.e. i*tanh(g) with the g-gate pre-scaled by 2
    on the host) -> DVE tensor_tensor_scan computing c = f*c_prev + ig in one
    instruction -> ACT tanh -> DVE h = o*tanh(c) -> next MM.

    The scan streams interleaved pairs: state = (d0*state)+d1 over
    [0,f_j] x [c_prev_j, ig_j]; the 0-multiply resets the state at each batch
    boundary, so one 16-element instruction computes all 8 independent
    f*c_prev+ig updates.  All cell operands live in one persistent SBUF tile
    `cm` laid out so every scan operand is a CONTIGUOUS 2D slice:
      cols 0:64   sigmoid out at stride 2: i@0+2j, f@16+2j, g@32+2j, o@48+2j;
                  odd cols stay zero (memset once) = the scan reset zeros
      cols 64:81  scan out, even steps: [echo@64+2j | c@65+2j]; odd steps' IG
                  overwrites the echoes shifted by one: ig@66+2j
      cols 81:98  scan out, odd steps: [echo@81+2j | c@82+2j]; even steps'
                  ig@83+2j
    data0 = cm[15:31] = [0,f0,0,f1,...]; data1(p) = [c0,ig0,c1,ig1,...].
    """
    nc = tc.nc
    B_L = B_LOC
    GB = G * B_L
    NW = S // T_W
    assert S % T_W == 0 and GB * T_W == 512

    x, whg, wxg, bg, wd, bd, yT = (io[k] for k in ("x", "whg", "wxg", "bg", "wd", "bd", "yT"))

    wpool = ctx.enter_context(tc.tile_pool(name="weights", bufs=1))
    dpool = ctx.enter_context(tc.tile_pool(name="dve", bufs=8))
    xspool = ctx.enter_context(tc.tile_pool(name="xstg", bufs=2))
    xbpool = ctx.enter_context(tc.tile_pool(name="xbf", bufs=3))
    hpool = ctx.enter_context(tc.tile_pool(name="hwin", bufs=6))

    wh_sb = wpool.tile([128, 4 * H], BF16, tag="wh")
    nc.sync.dma_start(wh_sb[:], whg[:])
    wx_sb = wpool.tile([64, 4 * H], BF16, tag="wx")
    nc.sync.dma_start(wx_sb[:], wxg[:])
    b4_sb = wpool.tile([G, H], BF16, tag="b4")
    nc.sync.dma_start(b4_sb[:], io["bg4"][:])
    e4_sb = wpool.tile([G, G * B_L], BF16, tag="e4")
    nc.sync.dma_start(e4_sb[:], io["e4"][:])
    wd_sb = wpool.tile([128, 1], BF16, tag="wd")
    nc.sync.dma_start(wd_sb[:], wd[:])
    bd_sb = wpool.tile([1, 1], F32, tag="bd")
    nc.sync.dma_start(bd_sb[:], bd[:])
    ones_sb = wpool.tile([1, 512], BF16, tag="ones")
    nc.vector.memset(ones_sb[:], 1.0)
    zrow_sb = wpool.tile([1, 128], BF16, tag="zrow")
    nc.vector.memset(zrow_sb[:], 0.0)

    # persistent cell memory (layout in docstring)
    cm = wpool.tile([128, 100], F32, tag="cm")
    nc.vector.memset(cm[:, 0:100], 0.0)

    if CELL_MODE == "scan":
        sig_out = cm[:, 0:64].rearrange("p (j e) -> p j e", e=2)[:, :, 0]
        si_ap = cm[:, 0:16].rearrange("p (j e) -> p j e", e=2)[:, :, 0]
        sg_ap = cm[:, 32:48].rearrange("p (j e) -> p j e", e=2)[:, :, 0]
        so_ap = cm[:, 48:64].rearrange("p (j e) -> p j e", e=2)[:, :, 0]
        d0 = cm[:, 15:31]                                 # [0,f0,0,f1,...]
        d1 = [cm[:, 82:98], cm[:, 65:81]]                 # [c_prev|ig] per parity
        ig_out = [cm[:, 83:99].rearrange("p (j e) -> p j e", e=2)[:, :, 0],
                  cm[:, 66:82].rearrange("p (j e) -> p j e", e=2)[:, :, 0]]
        c_out = [cm[:, 64:80], cm[:, 81:97]]
        c_odd = [cm[:, 65:81].rearrange("p (j e) -> p j e", e=2)[:, :, 0],
                 cm[:, 82:98].rearrange("p (j e) -> p j e", e=2)[:, :, 0]]
    else:  # 'mul3': contiguous layout, 3-op cell (ig, f*c, add)
        sig_out = cm[:, 0:32]
        si_ap, sf_ap = cm[:, 0:8], cm[:, 8:16]
        sg_ap, so_ap = cm[:, 16:24], cm[:, 24:32]
        wv_ap = cm[:, 32:40]
        fv_ap = cm[:, 40:48]
        c_t = [cm[:, 48:56], cm[:, 56:64]]

    # x windows flow in as ONE plain contiguous DMA per window (f32), are
    # transposed batch-by-batch on the tensor engine (identity-matmul into a
    # PSUM scratch bank) and cast to bf16 by a single DVE copy.  No
    # conversion DMAs, no xbar transposes; everything is off the critical
    # path and software-pipelined one window ahead.
    ident = io["ident"]
    id_sb = wpool.tile([T_W, T_W], F32, tag="ident")
    nc.sync.dma_start(id_sb[:], ident[:])
    xw_tiles = {}

    zw_tiles = {}
    hwin_tiles = {}

    with tc.tile_pool(name="psum", bufs=3, space="PSUM") as ppool:

        def t3(w):
            # t-major window layout: column = tl*GB + g*B_L + b, so every
            # per-step matmul output and the sigmoid input are CONTIGUOUS
            return zw_tiles[w][:].rearrange("p (t r) -> p t r", r=GB)

        def xprep_dma(w):
            xs = xspool.tile([T_W, B_L * D], F32, tag="xs", name=f"xs_{w}")
            nc.sync.dma_start(
                xs[:].rearrange("t (b d) -> t b d", d=D),
                x[:, w * T_W:(w + 1) * T_W, :].rearrange("b t d -> t b d"))
            xw_tiles[w] = [xs, None, None]

        def xprep_transpose(w):
            xs = xw_tiles[w][0]
            xtp = ppool.tile([64, B_L * T_W], F32, tag="xtp", name=f"xtp_{w}", bufs=2)
            for b in range(B_L):
                nc.tensor.transpose(xtp[:, T_W * b:T_W * (b + 1)],
                                    xs[:, D * b:D * (b + 1)], id_sb[:])
            xw_tiles[w][1] = xtp

        def xprep_cast(w):
            xbf = xbpool.tile([64, B_L * T_W], BF16, tag="xb", name=f"xb_{w}")
            nc.vector.tensor_copy(xbf[:], xw_tiles[w][1][:])
            xw_tiles[w][2] = xbf

        def prefill(w):
            zw_tiles[w] = ppool.tile([128, 512], F32, tag="zw", name=f"zw_{w}")
            hwin_tiles[w] = hpool.tile([128, T_W * B_L], BF16, tag="hw", name=f"hw_{w}")

        def prefill_zero(w):
            nc.tensor.matmul(zw_tiles[w][:, 0:512], zrow_sb[:], ones_sb[:],
                             start=True, stop=False, skip_group_check=True)

        def prefill_mms(w, ts):
            zt3 = t3(w)
            xbf3 = xw_tiles[w][2][:].rearrange("p (b t) -> p b t", t=T_W)
            for tp in ts:
                # bias for all 4 gates in one K=4 matmul: B4[g,:] x E4[g,col]
                nc.tensor.matmul(zt3[:, tp, :], b4_sb[:], e4_sb[:],
                                 start=False, stop=False, skip_group_check=True)
                for g in range(G):
                    nc.tensor.matmul(
                        zt3[:, tp, g * B_L:(g + 1) * B_L],
                        wx_sb[:, H * g:H * (g + 1)],
                        xbf3[:, :, tp],
                        start=False, stop=False, skip_group_check=True)

        hc_pool = ctx.enter_context(tc.tile_pool(name="hc", bufs=2))
        WPC = 512 // (T_W * B_L)

        def head_chunk(k):
            yp = ppool.tile([1, 512], F32, tag="yc", name=f"yc_{k}", bufs=1)
            ncol = T_W * B_L
            nc.tensor.matmul(yp[:], zrow_sb[0:1, 0:1], ones_sb[:],
                             start=True, stop=False, skip_group_check=True)
            for j in range(WPC):
                nc.tensor.matmul(yp[:, ncol * j:ncol * (j + 1)],
                                 wd_sb[:], hwin_tiles[WPC * k + j][:],
                                 start=False, stop=True, skip_group_check=True)
            ys = hc_pool.tile([1, 512], F32, tag="ys", name=f"ys_{k}")
            nc.scalar.activation(ys[:], yp[:], AF.Sigmoid, bias=bd_sb[0:1, 0:1])
            nc.sync.dma_start(yT[0:1, 512 * k:512 * (k + 1)], ys[:])

        xprep_dma(0)
        xprep_transpose(0)
        xprep_cast(0)
        prefill(0)
        prefill_zero(0)
        prefill_mms(0, ts=range(T_W))

        for t in range(S):
            w, tl = divmod(t, T_W)

            zt3 = t3(w)
            if t > 0:
                hsl_prev = hwin_tiles[divmod(t - 1, T_W)[0]][
                    :, divmod(t - 1, T_W)[1] * B_L:(divmod(t - 1, T_W)[1] + 1) * B_L]
                for g in range(G):
                    nc.tensor.matmul(
                        zt3[:, tl, g * B_L:(g + 1) * B_L],
                        wh_sb[:, H * g:H * (g + 1)],
                        hsl_prev,
                        start=False, stop=True, skip_group_check=True)
            # sigmoid over all four gates
            nc.scalar.activation(sig_out, zt3[:, tl, 0:GB], AF.Sigmoid)
            p = t & 1
            if CELL_MODE == "scan":
                # ig = 2*si*sg - si, interleaved into the prev scan-out region
                nc.vector._custom_dve(get_ig_op(), out=ig_out[p],
                                      in0=si_ap, in1=sg_ap, s0=2.0, s1=1.0)
                # c = f*c_prev + ig in one scan: state=(d0*state)+d1 over pairs
                nc.vector.tensor_tensor_scan(
                    c_out[p], d0, d1[p], 0.0, OP.mult, OP.add)
                c_ap = c_odd[p]
            else:
                nc.vector._custom_dve(get_ig_op(), out=wv_ap,
                                      in0=si_ap, in1=sg_ap, s0=2.0, s1=1.0)
                nc.vector.tensor_tensor(fv_ap, sf_ap, c_t[1 - p], OP.mult)
                nc.vector.tensor_tensor(c_t[p], fv_ap, wv_ap, OP.add)
                c_ap = c_t[p]
            # tanh(c) then h = o*tanh(c) straight into the matmul rhs window
            th = dpool.tile([128, B_L], F32, tag="th", name=f"th_{t}")
            nc.scalar.activation(th[:], c_ap, AF.Tanh)
            hsl = hwin_tiles[w][:, tl * B_L:(tl + 1) * B_L]
            nc.vector.tensor_tensor(hsl, so_ap, th[:], OP.mult)

            # auxiliary work is emitted AFTER the step's recurrence ops so it
            # lands in each in-order engine queue behind the critical ops and
            # executes in their idle windows
            if w + 1 < NW:
                if tl == 0:
                    xprep_dma(w + 1)
                elif tl == 4:
                    xprep_transpose(w + 1)
                elif tl == 6:
                    xprep_cast(w + 1)
                elif tl == 7:
                    prefill(w + 1)
                    prefill_zero(w + 1)
                elif tl >= 8:
                    prefill_mms(w + 1, ts=[2 * (tl - 8), 2 * (tl - 8) + 1])
            if HEAD_INTERLEAVE and tl == 10 and w >= WPC and (w % WPC == 0):
                head_chunk(w // WPC - 1)

        nchunks = S * B_L // 512
        done = len([w for w in range(WPC, NW) if w % WPC == 0]) if HEAD_INTERLEAVE else 0
        for k in range(done, nchunks):
            head_chunk(k)


def build_nc(S=S_FULL, n_chains=N_CHAINS):
    nc = bacc.Bacc("TRN2", target_bir_lowering=False, debug=False)
    io = {
        "x": nc.dram_tensor("x", [B_LOC, S, D], F32, kind="ExternalInput").ap(),
        "whg": nc.dram_tensor("whg", [H, 4 * H], BF16, kind="ExternalInput").ap(),
        "wxg": nc.dram_tensor("wxg", [D, 4 * H], BF16, kind="ExternalInput").ap(),
        "bg": nc.dram_tensor("bg", [1, 4 * H], BF16, kind="ExternalInput").ap(),
        "wd": nc.dram_tensor("wd", [H, 1], BF16, kind="ExternalInput").ap(),
        "bd": nc.dram_tensor("bd", [1, 1], F32, kind="ExternalInput").ap(),
        "ident": nc.dram_tensor("ident", [T_W, T_W], F32, kind="ExternalInput").ap(),
        "bg4": nc.dram_tensor("bg4", [G, H], BF16, kind="ExternalInput").ap(),
        "e4": nc.dram_tensor("e4", [G, G * B_LOC], BF16, kind="ExternalInput").ap(),
        "yT": nc.dram_tensor("yT", [1, S * B_LOC], F32, kind="ExternalOutput").ap(),
    }
    with tile.TileContext(nc) as tc:
        with ExitStack() as ctx:
            if n_chains == 0:  # v2: fused-cell single chain
                emit_lstm_v2(ctx, tc, io, S=S)
            else:
                emit_lstm(ctx, tc, io, S=S, n_chains=n_chains)
    nc.compile()
    fn = nc.m.functions[0]
    strip_act_evsems(fn)
    strip_same_engine_waits(fn)
    strip_transitive_pe_waits(fn)
    fold_single_wait_evsems(fn)
    return nc


_CACHE = {}


def _get_compiled():
    if "nc" not in _CACHE:
        _CACHE["nc"] = build_nc()
    return _CACHE["nc"]


def _get_fast_runner():
    """Stable jitted SPMD callable, built once; subsequent kernel() calls only
    pay input transfer + dispatch + execute."""
    if "fast_run" in _CACHE:
        return _CACHE["fast_run"]

    import jax
    from jax.experimental.shard_map import shard_map
    from jax.sharding import Mesh, PartitionSpec

    from concourse import bass2jax

    nc = _get_compiled()
    bass2jax.install_neuronx_cc_hook()
    partition_name = nc.partition_id_tensor.name if nc.partition_id_tensor else None

    in_names, out_names, out_avals, zero_outs = [], [], [], []
    for alloc in nc.m.functions[0].allocations:
        if not isinstance(alloc, mybir.MemoryLocationSet):
            continue
        name = alloc.memorylocations[0].name
        if alloc.kind == "ExternalInput":
            if name != partition_name:
                in_names.append(name)
        elif alloc.kind == "ExternalOutput":
            out_names.append(name)
            shape = tuple(alloc.tensor_shape)
            dtype = mybir.dt.np(alloc.dtype)
            out_avals.append(jax.core.ShapedArray(shape, dtype))
            zero_outs.append(np.zeros((NCORES * shape[0], *shape[1:]), dtype))
    n_params = len(in_names)
    all_names = in_names + out_names
    if partition_name is not None:
        all_names = all_names + [partition_name]

    def _body(*args):
        operands = list(args)
        if partition_name is not None:
            operands.append(bass2jax.partition_id_tensor())
        outs = bass2jax._bass_exec_p.bind(
            *operands,
            out_avals=tuple(out_avals),
            in_names=tuple(all_names),
            out_names=tuple(out_names),
            lowering_input_output_aliases=(),
            sim_require_finite=True,
            sim_require_nnan=True,
            nc=nc,
        )
        return tuple(outs)

    devices = jax.devices()[:NCORES]
    mesh = Mesh(np.asarray(devices), ("core",))
    donate = tuple(range(n_params, n_params + len(out_names)))
    sharded = jax.jit(
        shard_map(_body, mesh=mesh,
                  in_specs=(PartitionSpec("core"),) * (n_params + len(out_names)),
                  out_specs=(PartitionSpec("core"),) * len(out_names),
                  check_rep=False),
        donate_argnums=donate, keep_unused=True)

    def run(concat_by_name):
        ins = [concat_by_name[k] for k in in_names]
        zeros = [z.copy() for z in zero_outs]  # donated each call
        out_arrs = sharded(*ins, *zeros)
        return {k: np.asarray(out_arrs[i]) for i, k in enumerate(out_names)}

    _CACHE["fast_run"] = run
    return run


def kernel(**inputs):
    x = np.ascontiguousarray(np.asarray(inputs["x"], np.float32))
    w = prep_weights(inputs["Wx"], inputs["Wh"], inputs["b"], inputs["Wd"], inputs["bd"])
    run = _get_fast_runner()
    # shard_map splits axis 0 across the 8 cores: per-core x shard
    # [B_LOC, S, D] concatenated along axis 0 is exactly the full x.
    concat = {"x": x}
    for k, v in w.items():
        concat[k] = np.concatenate([v] * NCORES, axis=0)
    outs = run(concat)
    yt = outs["yT"].reshape(NCORES, S_FULL, B_LOC)
    y = np.zeros((B, S_FULL, 1), np.float32)
    for c in range(NCORES):
        y[c * B_LOC:(c + 1) * B_LOC, :, 0] = yt[c].T
    return y


# ---------------------------------------------------------------------------
# Stable-jit SPMD runner (mirrors bass_utils.run_bass_kernel_spmd's axon path
# but keeps one jitted callable so repeated runs don't recompile).

def make_runner(nc, n_cores=NCORES):
    import jax
    from jax.experimental.shard_map import shard_map
    from jax.sharding import Mesh, PartitionSpec

    from concourse import bass2jax

    bass2jax.install_neuronx_cc_hook()
    assert nc.dbg_addr is None
    partition_name = nc.partition_id_tensor.name if nc.partition_id_tensor else None

    in_names, out_names, out_avals, zero_outs = [], [], [], []
    for alloc in nc.m.functions[0].allocations:
        if not isinstance(alloc, mybir.MemoryLocationSet):
            continue
        name = alloc.memorylocations[0].name
        if alloc.kind == "ExternalInput":
            if name != partition_name:
                in_names.append(name)
        elif alloc.kind == "ExternalOutput":
            out_names.append(name)
            shape = tuple(alloc.tensor_shape)
            dtype = mybir.dt.np(alloc.dtype)
            out_avals.append(jax.core.ShapedArray(shape, dtype))
            zero_outs.append(np.zeros(shape, dtype))
    n_params = len(in_names)
    all_names = in_names + out_names
    if partition_name is not None:
        all_names = all_names + [partition_name]

    def _body(*args):
        operands = list(args)
        if partition_name is not None:
            operands.append(bass2jax.partition_id_tensor())
        outs = bass2jax._bass_exec_p.bind(
            *operands,
            out_avals=tuple(out_avals),
            in_names=tuple(all_names),
            out_names=tuple(out_names),
            lowering_input_output_aliases=(),
            sim_require_finite=True,
            sim_require_nnan=True,
            nc=nc,
        )
        return tuple(outs)

    devices = jax.devices()[:n_cores]
    mesh = Mesh(np.asarray(devices), ("core",))
    donate = tuple(range(n_params, n_params + len(out_names)))
    sharded = jax.jit(
        shard_map(_body, mesh=mesh,
                  in_specs=(PartitionSpec("core"),) * (n_params + len(out_names)),
                  out_specs=(PartitionSpec("core"),) * len(out_names),
                  check_rep=False),
        donate_argnums=donate, keep_unused=True)

    def run(in_maps):
        concat_in = [np.concatenate([np.asarray(in_maps[c][k]) for c in range(n_cores)], axis=0)
                     for k in in_names]
        concat_zero = [np.zeros((n_cores * z.shape[0], *z.shape[1:]), z.dtype) for z in zero_outs]
        out_arrs = sharded(*concat_in, *concat_zero)
        return [
            {k: np.asarray(out_arrs[i]).reshape(n_cores, *out_avals[i].shape)[c]
             for i, k in enumerate(out_names)}
            for c in range(n_cores)
        ]

    return run


def make_null_nc(S=S_FULL):
    """Same external IO signature as the LSTM kernel, but only a token DMA —
    for calibrating per-call dispatch overhead in timing runs."""
    nc = bacc.Bacc("TRN2", target_bir_lowering=False, debug=False)
    x = nc.dram_tensor("x", [B_LOC, S, D], F32, kind="ExternalInput").ap()
    nc.dram_tensor("whg", [H, 4 * H], BF16, kind="ExternalInput").ap()
    nc.dram_tensor("wxg", [D, 4 * H], BF16, kind="ExternalInput").ap()
    nc.dram_tensor("bg", [1, 4 * H], BF16, kind="ExternalInput").ap()
    nc.dram_tensor("wd", [H, 1], BF16, kind="ExternalInput").ap()
    nc.dram_tensor("bd", [1, 1], F32, kind="ExternalInput").ap()
    yT = nc.dram_tensor("yT", [1, S * B_LOC], F32, kind="ExternalOutput").ap()
    with tile.TileContext(nc) as tc:
        with tc.tile_pool(name="p", bufs=1) as p:
            t = p.tile([1, 512], F32, name="tnull")
            nc.sync.dma_start(t[:], x[0, 0:8, 0:64].rearrange("a b -> (a b)")[None, :])
            nc.sync.dma_start(yT[0:1, 0:512], t[:])
    nc.compile()
    return nc


def make_device_runner(nc, n_cores=NCORES, n_zero_sets=12):
    """Like make_runner but with inputs pre-placed on device; returns
    (prepare(in_maps) -> None, run_once() -> outs) for tight timing loops."""
    import jax
    from jax.experimental.shard_map import shard_map
    from jax.sharding import Mesh, NamedSharding, PartitionSpec

    from concourse import bass2jax

    bass2jax.install_neuronx_cc_hook()
    partition_name = nc.partition_id_tensor.name if nc.partition_id_tensor else None
    in_names, out_names, out_avals, zero_outs = [], [], [], []
    for alloc in nc.m.functions[0].allocations:
        if not isinstance(alloc, mybir.MemoryLocationSet):
            continue
        name = alloc.memorylocations[0].name
        if alloc.kind == "ExternalInput":
            if name != partition_name:
                in_names.append(name)
        elif alloc.kind == "ExternalOutput":
            out_names.append(name)
            shape = tuple(alloc.tensor_shape)
            dtype = mybir.dt.np(alloc.dtype)
            out_avals.append(jax.core.ShapedArray(shape, dtype))
            zero_outs.append(np.zeros(shape, dtype))
    n_params = len(in_names)
    all_names = in_names + out_names
    if partition_name is not None:
        all_names = all_names + [partition_name]

    def _body(*args):
        operands = list(args)
        if partition_name is not None:
            operands.append(bass2jax.partition_id_tensor())
        outs = bass2jax._bass_exec_p.bind(
            *operands,
            out_avals=tuple(out_avals),
            in_names=tuple(all_names),
            out_names=tuple(out_names),
            lowering_input_output_aliases=(),
            sim_require_finite=True,
            sim_require_nnan=True,
            nc=nc,
        )
        return tuple(outs)

    devices = jax.devices()[:n_cores]
    mesh = Mesh(np.asarray(devices), ("core",))
    donate = tuple(range(n_params, n_params + len(out_names)))
    sharded = jax.jit(
        shard_map(_body, mesh=mesh,
                  in_specs=(PartitionSpec("core"),) * (n_params + len(out_names)),
                  out_specs=(PartitionSpec("core"),) * len(out_names),
                  check_rep=False),
        donate_argnums=donate, keep_unused=True)
    shard = NamedSharding(mesh, PartitionSpec("core"))

    state = {}

    def prepare(in_maps):
        concat_in = [np.concatenate([np.asarray(in_maps[c][k]) for c in range(n_cores)], axis=0)
                     for k in in_names]
        state["dev_in"] = [jax.device_put(a, shard) for a in concat_in]
        state["zeros"] = [
            [jax.device_put(np.zeros((n_cores * z.shape[0], *z.shape[1:]), z.dtype), shard)
             for z in zero_outs]
            for _ in range(n_zero_sets)
        ]
        state["k"] = 0

    def run_once():
        zs = state["zeros"][state["k"] % len(state["zeros"])]
        state["k"] += 1
        out = sharded(*state["dev_in"], *zs)
        jax.block_until_ready(out)
        return out

    return prepare, run_once

